# revision 1
# baseline (speedup 1.0000x reference)
"""DispMVS depth-fusion kernel for 8 Trainium2 NeuronCores.

Sharding: core c handles batch b = c // 4 and coarse rows r0 = (c % 4) * 64
(64 of 256 rows), with BOTH neighbor streams (NN=2) resident on the core
(partitions = nn*64 + row).  The cross-neighbor confidence-fusion softmax is
then core-local; cores never communicate.

Pipeline per core (one Bass/Tile program, identical for all 8 cores):
  1. geometry: elementwise epipolar math -> clipped inverse depth  [128, 330]
     (330 = 320 cols + 10 packed halo pixels/partition for the rows just
     outside the chunk, needed by the 3x3 unfold).
  2. DRAM scratch round-trip to rebuild inv-depth as 3 vertically shifted,
     zero-padded rows per partition (the unfold operand); conf comes the same
     way directly from a host-padded input.
  3. per (p, w-half) chunk: exp(mask) on ACT, grouped 9-way reductions on DVE
     (softmax numerators/denominator), convex-upsample of inv-depth and conf,
     then the 2-view softmax fusion and final reciprocal.
"""

import numpy as np

NN, B, H, W = 2, 2, 256, 320
UP = 4
EPS = 1e-6
RPC = 64          # coarse rows per core
NCORES = 8
HW = H * W
RW = RPC * W      # elements in one [64, 320] channel-slice

# consts columns
(
    C_M00, C_M01, C_M02, C_M10, C_M11, C_M12, C_M20, C_M21, C_M22,
    C_T0, C_T1, C_T2,
    C_R00, C_R01, C_R02, C_R10, C_R11, C_R12, C_R20, C_R21, C_R22,
    C_A0, C_A1, C_A2, C_B0, C_B1, C_B2,
    C_TX, C_TY, C_TZ,
    C_CA, C_CB, C_DS, C_DB, C_TEN,
) = range(35)
NCONST = 36

_cache = {}


def _register_custom_ops():
    """Register this kernel's custom DVE ops (idempotent). Returns a dict.

    MUL_CUMSUM_ANT: out = cumsum(in0*in1) along the free stream - grouped
      9-tap weighted sums fall out as differences of every-9th prefix value,
      one line-rate pass instead of multiply + strided TENSOR_REDUCE passes.
    SUMSQ_ANT: out = in0^2 + in1^2 (one pass instead of 3).
    RSQRT_NR_ANT: one Newton step for 1/sqrt: out = in0*(1.5 - 0.5*in1*in0^2)
      (one pass instead of 4).
    """
    from concourse import dve_ops
    from concourse.dve_spec import AluOp, C0, C1, Spec, Src0, Src1, _has_src1, lower, scan
    from concourse.dve_uop import DveOpSpec

    have = {o.name: o for o in dve_ops.OPS}
    if "MUL_CUMSUM_ANT" in have:
        return have

    def cum_ref(in0, in1, s0, s1, imm2):
        a = in0.astype(np.float32).reshape(in0.shape[0], -1) * in1.astype(
            np.float32
        ).reshape(in1.shape[0], -1)
        return np.cumsum(a, axis=1, dtype=np.float32).reshape(in0.shape)

    specs = [
        ("MUL_CUMSUM_ANT", Spec(body=scan(AluOp.ADD, Src0 * Src1), reference=cum_ref)),
        (
            "SUMSQ_ANT",
            Spec(
                body=Src0 * Src0 + Src1 * Src1,
                reference=lambda in0, in1, s0, s1, imm2: (
                    in0.astype(np.float32) ** 2 + in1.astype(np.float32) ** 2
                ),
            ),
        ),
        (
            "RSQRT_NR_ANT",
            Spec(
                body=(Src0 * Src0 * Src1 * C0 + C1) * Src0,
                reference=lambda in0, in1, s0, s1, imm2: (
                    (in0.astype(np.float32) ** 2 * in1 * s0 + s1) * in0
                ),
            ),
        ),
    ]
    out = {}
    for name, spec in specs:
        op = dve_ops.DveOp(name, spec, subdim=False, uops_sha={})
        dve_ops.OPS.append(op)
        dve_ops.CUSTOM_DVE_SPECS[name] = spec
        dve_ops._SUB_OPCODE_FOR_NAME[name] = (
            dve_ops._CUSTOM_DVE_ROW_BASE + len(dve_ops.OPS) - 1
        )
        for ver in ("v3", "v4"):
            tmp = DveOpSpec(
                name=name,
                opcode=dve_ops.get_dve_sub_opcode(name),
                uops=lower(spec, ver=ver),
                rd1_en=_has_src1(spec),
            )
            op.uops_sha[ver] = tmp.sha(ver)
        out[name] = op
    assert max(dve_ops._SUB_OPCODE_FOR_NAME.values()) < 0x20
    return out


def _build_program():
    import concourse.bass as bass
    import concourse.bacc as bacc
    import concourse.tile as tile
    from concourse import mybir
    from concourse.alu_op_type import AluOpType as op

    f32 = mybir.dt.float32
    i32 = mybir.dt.int32
    Act = mybir.ActivationFunctionType

    cops = _register_custom_ops()
    nc = bacc.Bacc("TRN2", target_bir_lowering=False, debug=False)

    pix_d = nc.dram_tensor("pix", [128, 4, 330], f32, kind="ExternalInput").ap()
    consts_d = nc.dram_tensor("consts", [128, NCONST], f32, kind="ExternalInput").ap()
    hm_d = nc.dram_tensor("hm", [128, 10], f32, kind="ExternalInput").ap()
    confpad_d = nc.dram_tensor("confpad", [NN, 66, 322], f32, kind="ExternalInput").ap()
    # mask pre-packed on host to [gc, wc, (nn,r), q, w, k] (k innermost) so each
    # chunk's DMA is one contiguous [128, 5760] transfer and the 9-tap groups
    # are unit-stride for the cumsum trick
    mask_d = nc.dram_tensor("maskpk", [4, 2, 128, 4, 160, 9], f32, kind="ExternalInput").ap()
    scr = nc.dram_tensor("scr", [NN, 66, 322], f32, kind="Internal").ap()
    out_d = nc.dram_tensor("out", [RPC * UP, W * UP], f32, kind="ExternalOutput").ap()

    def dram_ap(base, off, dims):
        return bass.AP(tensor=base.tensor, offset=base.offset + off, ap=[list(d) for d in dims])

    with tile.TileContext(nc) as tc:
        with tc.tile_pool(name="persist", bufs=1) as pp:
            ep_ctx = tc.tile_pool(name="early", bufs=1)
            ep = ep_ctx.__enter__()
            consts = pp.tile([128, NCONST], f32, name="consts")
            nc.sync.dma_start(out=consts[:], in_=consts_d)

            def CC(i, p0=0, p1=128):
                return consts[p0:p1, i : i + 1]

            pix = ep.tile([128, 4, 330], f32, name="pix")
            nc.sync.dma_start(out=pix[:], in_=pix_d)
            hm = ep.tile([128, 10], f32, name="hm")
            nc.sync.dma_start(out=hm[:], in_=hm_d)

            t3i = ep.tile([128, 3, 322], f32, name="t3i")  # unfold rows of inv-depth
            t3c = ep.tile([128, 3, 322], f32, name="t3c")  # unfold rows of conf
            # conf unfold rows straight from the host-padded input
            for nn in range(NN):
                src = dram_ap(
                    confpad_d, nn * 66 * 322,
                    [[322, 64], [322, 3], [1, 322]],
                )
                nc.sync.dma_start(out=t3c[nn * 64 : nn * 64 + 64], in_=src)

            inv_res = ep.tile([128, 330], f32, name="inv_res")
            zero2 = ep.tile([2, 132], f32, name="zero2")
            nc.vector.memset(zero2[:], 0.0)

            # ---------------- geometry ----------------
            u = pix[:, 0, :]
            v = pix[:, 1, :]
            d = pix[:, 2, :]
            fl = pix[:, 3, :]

            with tc.tile_pool(name="geom", bufs=1) as gp:
                _tagn = [0]

                def T(shape=(128, 330)):
                    _tagn[0] += 1
                    return gp.tile(list(shape), f32, name=f"g{_tagn[0]}", tag=f"g{_tagn[0]}")

                def TT(o, a, b, alu):
                    nc.vector.tensor_tensor(out=o, in0=a, in1=b, op=alu)

                def TS(o, a, s1, o0, s2=None, o1=None):
                    if o1 is None:
                        nc.vector.tensor_scalar(out=o, in0=a, scalar1=s1, scalar2=None, op0=o0)
                    else:
                        nc.vector.tensor_scalar(out=o, in0=a, scalar1=s1, scalar2=s2, op0=o0, op1=o1)

                def STT(o, a, s, b, o0, o1):
                    nc.vector.scalar_tensor_tensor(out=o, in0=a, scalar=s, in1=b, op0=o0, op1=o1)

                def AB(o, a):
                    nc.scalar.activation(out=o, in_=a, func=Act.Abs)

                def AF(o, a, scale, bias):
                    nc.scalar.activation(out=o, in_=a, func=Act.Identity, scale=scale, bias=bias)

                def recip_acc(o, x):
                    t = T()
                    nc.vector.reciprocal_approx_accurate(out=o, in_=x, scratch=t[:])

                # a_j = M @ [u, v, 1]
                a0, a1, a2 = T(), T(), T()
                tmp = T()
                AF(tmp[:], u, CC(C_M00), CC(C_M02))
                STT(a0[:], v, CC(C_M01), tmp[:], op.mult, op.add)
                AF(tmp[:], u, CC(C_M10), CC(C_M12))
                STT(a1[:], v, CC(C_M11), tmp[:], op.mult, op.add)
                AF(tmp[:], u, CC(C_M20), CC(C_M22))
                STT(a2[:], v, CC(C_M21), tmp[:], op.mult, op.add)

                d10 = T()
                AF(d10[:], d, 1.0, CC(C_TEN))

                # z components and their reciprocals
                ps2, pe2, rs2, re2 = T(), T(), T(), T()
                m = T()
                TT(m[:], a2[:], d, op.mult)
                AF(ps2[:], m[:], 1.0, CC(C_T2))
                TT(m[:], a2[:], d10[:], op.mult)
                TT(pe2[:], m[:], ps2[:], op.add)
                AB(m[:], ps2[:])
                TS(m[:], m[:], EPS, op.add)
                recip_acc(rs2[:], m[:])
                AB(m[:], pe2[:])
                TS(m[:], m[:], EPS, op.add)
                recip_acc(re2[:], m[:])

                # x/y components, start and end projections
                pxs, pys, pxe, pye = T(), T(), T(), T()
                for aj, tj, po_s, po_e in ((a0, C_T0, pxs, pxe), (a1, C_T1, pys, pye)):
                    psj, pej = T((128, 330)), T((128, 330))
                    TT(m[:], aj[:], d, op.mult)
                    AF(psj[:], m[:], 1.0, CC(tj))
                    TT(m[:], aj[:], d10[:], op.mult)
                    TT(pej[:], m[:], psj[:], op.add)
                    TT(po_s[:], psj[:], rs2[:], op.mult)
                    TT(po_e[:], pej[:], re2[:], op.mult)

                fdx, fdy = T(), T()
                TT(fdx[:], pxe[:], pxs[:], op.subtract)
                TT(fdy[:], pye[:], pys[:], op.subtract)

                # rsqrt(fdx^2 + fdy^2) via magic seed + 2 fused Newton steps
                q = T()
                nc.vector._custom_dve(cops["SUMSQ_ANT"], out=q[:], in0=fdx[:], in1=fdy[:])
                y = T()
                yi = y[:].bitcast(i32)
                TS(yi, q[:].bitcast(i32), 1, op.arith_shift_right)
                TS(yi, yi, -1, op.bitwise_xor)
                TS(yi, yi, 0x5F3759DF + 1, op.add)
                y2 = T()
                nc.vector._custom_dve(
                    cops["RSQRT_NR_ANT"], out=y2[:], in0=y[:], in1=q[:], s0=-0.5, s1=1.5
                )
                nc.vector._custom_dve(
                    cops["RSQRT_NR_ANT"], out=y[:], in0=y2[:], in1=q[:], s0=-0.5, s1=1.5
                )

                fls = T()
                TT(fls[:], fl, y[:], op.mult)
                mx, my = T(), T()
                TT(m[:], fdx[:], fls[:], op.mult)
                TT(mx[:], m[:], pxs[:], op.add)
                TT(m[:], fdy[:], fls[:], op.mult)
                TT(my[:], m[:], pys[:], op.add)

                fm = T()
                fmi = fm[:].bitcast(i32)
                ax = T()
                AB(ax[:], fdx[:])
                AB(m[:], fdy[:])
                TT(fmi, m[:], ax[:], op.is_gt)

                nx, ny = T(), T()
                AF(tmp[:], mx[:], CC(C_A0), CC(C_A2))
                STT(nx[:], my[:], CC(C_A1), tmp[:], op.mult, op.add)
                AF(tmp[:], mx[:], CC(C_B0), CC(C_B2))
                STT(ny[:], my[:], CC(C_B1), tmp[:], op.mult, op.add)

                rx, ry, rz = T(), T(), T()
                AF(tmp[:], u, CC(C_R00), CC(C_R02))
                STT(rx[:], v, CC(C_R01), tmp[:], op.mult, op.add)
                AF(tmp[:], u, CC(C_R10), CC(C_R12))
                STT(ry[:], v, CC(C_R11), tmp[:], op.mult, op.add)
                AF(tmp[:], u, CC(C_R20), CC(C_R22))
                STT(rz[:], v, CC(C_R21), tmp[:], op.mult, op.add)

                def inv_axis(o, nj, rj, c_t):
                    num = T()
                    TT(m[:], rz[:], nj[:], op.mult)
                    TT(m[:], rj[:], m[:], op.subtract)
                    AB(num[:], m[:])
                    AF(m[:], nj[:], CC(C_TZ), CC(c_t))
                    AB(m[:], m[:])
                    TS(m[:], m[:], EPS, op.add)
                    rden = T()
                    recip_acc(rden[:], m[:])
                    TT(o, num[:], rden[:], op.mult)

                invx, invy = T(), T()
                inv_axis(invx[:], nx, rx, C_TX)
                inv_axis(invy[:], ny, ry, C_TY)

                seld = T()
                nc.vector.select(out=seld[:], mask=fmi, on_true=invy[:], on_false=invx[:])
                AF(tmp[:], seld[:], CC(C_CA), CC(C_CB))
                TS(inv_res[:], tmp[:], 0.0, op.max, 1.0, op.min)

            # zero the halo pixels that fall outside the image (edge chunks)
            nc.vector.tensor_tensor(
                out=inv_res[:, 320:330], in0=inv_res[:, 320:330], in1=hm[:], op=op.mult
            )

            # ------- scratch round-trip: [nn, 66, 322] padded inv-depth -------
            for nn in range(NN):
                base = nn * 66 * 322
                sl = slice(nn * 64, nn * 64 + 64)
                nc.sync.dma_start(
                    out=dram_ap(scr, base + 322 + 1, [[322, 64], [1, 320]]),
                    in_=inv_res[sl, 0:320],
                )
                nc.sync.dma_start(
                    out=dram_ap(scr, base + 1, [[5, 64], [1, 5]]),
                    in_=inv_res[sl, 320:325],
                )
                nc.sync.dma_start(
                    out=dram_ap(scr, base + 65 * 322 + 1, [[5, 64], [1, 5]]),
                    in_=inv_res[sl, 325:330],
                )
                # zero pad columns 0 and 321 of all 66 rows
                nc.sync.dma_start(
                    out=dram_ap(scr, base, [[0, 1], [322, 66], [321, 2]]),
                    in_=zero2[nn : nn + 1, :].rearrange("p (a b) -> p a b", a=66),
                )
            for nn in range(NN):
                src = dram_ap(scr, nn * 66 * 322, [[322, 64], [322, 3], [1, 322]])
                nc.sync.dma_start(out=t3i[nn * 64 : nn * 64 + 64], in_=src)

            # unfold weights interleaved [w, k] (k innermost) so the
            # weighted-cumsum's src1 for any w-window is one contiguous slice
            ufi9i = pp.tile([128, 322, 9], f32, name="ufi9i")
            ufi9c = pp.tile([128, 322, 9], f32, name="ufi9c")
            for t3, ufi9 in ((t3i, ufi9i), (t3c, ufi9c)):
                for dy in range(3):
                    for dx in range(3):
                        nc.scalar.activation(
                            out=ufi9[:, 0 : 322 - dx, dy * 3 + dx],
                            in_=t3[:, dy, dx:322],
                            func=Act.Copy,
                        )

            ep_ctx.__exit__(None, None, None)

            # ---------------- upsample + fusion, 2 w-halves x 4 p-chunks ----------------
            WC = 160
            with tc.tile_pool(name="chunk", bufs=2) as cp, tc.tile_pool(
                name="chunk1", bufs=1
            ) as cp1:
                for wc in range(2):
                    w0 = wc * WC
                    ufs = {"i": ufi9i[:, w0 : w0 + WC, :], "c": ufi9c[:, w0 : w0 + WC, :]}
                    for gc in range(4):
                        e = cp.tile([128, 4, WC, 9], f32, name="e", tag="e")
                        nc.sync.dma_start(out=e[:], in_=mask_d[gc, wc])
                        nc.scalar.activation(out=e[:], in_=e[:], func=Act.Exp)

                        # softmax denominator: unit-stride innermost-k reduce
                        s = cp.tile([128, 4, WC], f32, name="s", tag="s")
                        nc.vector.tensor_reduce(
                            out=s[:], in_=e[:], axis=mybir.AxisListType.X, op=op.add
                        )
                        rs = cp.tile([128, 4, WC], f32, name="rs", tag="rs")
                        nc.vector.reciprocal_approx_fast(out=rs[:], in_=s[:])

                        up_t = {}
                        for tag in ("i", "c"):
                            cum = cp1.tile(
                                [128, 4, WC, 9], f32, name="cum", tag="cum", bufs=2
                            )
                            for g in range(4):
                                nc.vector._custom_dve(
                                    cops["MUL_CUMSUM_ANT"], out=cum[:, g], in0=e[:, g], in1=ufs[tag]
                                )
                            # every-9th prefix value, with a zero column prepended
                            ce = cp1.tile([128, 4, WC + 1], f32, name="ce", tag="ce" + tag)
                            nc.vector.memset(ce[:, :, 0:1], 0.0)
                            nc.scalar.activation(
                                out=ce[:, :, 1 : WC + 1], in_=cum[:, :, :, 8], func=Act.Copy
                            )
                            acc = cp.tile([128, 4, WC], f32, name="acc", tag="acc" + tag)
                            nc.vector.tensor_tensor(
                                out=acc[:],
                                in0=ce[:, :, 1 : WC + 1],
                                in1=ce[:, :, 0:WC],
                                op=op.subtract,
                            )
                            upv = cp.tile([128, 4, WC], f32, name="upv", tag="up" + tag)
                            nc.vector.tensor_tensor(out=upv[:], in0=acc[:], in1=rs[:], op=op.mult)
                            up_t[tag] = upv

                        iu, cu = up_t["i"], up_t["c"]
                        lo, hi = slice(0, 64), slice(64, 128)

                        def F(tag):
                            return cp.tile([64, 4, WC], f32, name="f" + tag, tag="f" + tag)

                        # TT operands must share a base partition: move the nn1
                        # halves down to partitions 0-63 via SBUF->SBUF DMA
                        iu2, cu2 = F("iu2"), F("cu2")
                        nc.sync.dma_start(out=iu2[:], in_=iu[hi])
                        nc.sync.dma_start(out=cu2[:], in_=cu[hi])

                        fa, fb, fc, fd = F("a"), F("b"), F("c"), F("d")
                        # fa=dif -> fb=exp(dif) -> fc=1+fb -> fd=1/fc
                        nc.vector.tensor_tensor(out=fa[:], in0=cu2[:], in1=cu[lo], op=op.subtract)
                        nc.scalar.activation(out=fb[:], in_=fa[:], func=Act.Exp)
                        nc.scalar.activation(out=fc[:], in_=fb[:], func=Act.Identity, bias=1.0)
                        nc.vector.reciprocal_approx_fast(out=fd[:], in_=fc[:])
                        # fa=iu1*e -> fc=fa+iu0 -> fa=fc*fd -> fc=scale*fa+bias
                        nc.vector.tensor_tensor(out=fa[:], in0=iu2[:], in1=fb[:], op=op.mult)
                        nc.vector.tensor_tensor(out=fc[:], in0=fa[:], in1=iu[lo], op=op.add)
                        nc.vector.tensor_tensor(out=fa[:], in0=fc[:], in1=fd[:], op=op.mult)
                        nc.scalar.activation(
                            out=fc[:], in_=fa[:], func=Act.Identity,
                            scale=CC(C_DS, 0, 64), bias=CC(C_DB, 0, 64),
                        )
                        out_t = cp.tile([64, WC, 4], f32, name="out_t", tag="out_t")
                        nc.vector.reciprocal_approx_fast(
                            out=out_t[:].rearrange("p w q -> p q w"), in_=fc[:]
                        )
                        dst = dram_ap(
                            out_d,
                            gc * (W * UP) + UP * w0,
                            [[UP * W * UP, 64], [UP, WC], [1, UP]],
                        )
                        nc.sync.dma_start(out=dst, in_=out_t[:])

    nc.finalize()
    return nc


def _host_prep(inputs):
    K_ref = np.asarray(inputs["K_ref"], np.float32)
    K_nei = np.asarray(inputs["K_nei"], np.float32)
    R_nei = np.asarray(inputs["R_nei"], np.float32)
    T_nei = np.asarray(inputs["T_nei"], np.float32)
    depth0 = np.asarray(inputs["depth0"], np.float32)
    flow = np.asarray(inputs["flow"], np.float32)
    mask = np.asarray(inputs["mask"], np.float32)
    conf = np.asarray(inputs["conf"], np.float32)
    dmin = float(np.asarray(inputs["depth_min"]).reshape(-1)[0])
    dmax = float(np.asarray(inputs["depth_max"]).reshape(-1)[0])

    # pixel rays per batch (u, v with unit z)
    uv = []
    for b in range(B):
        Ki = np.linalg.inv(K_ref[b, 0, 0].astype(np.float64))
        gx, gy = np.meshgrid(np.arange(W, dtype=np.float64), np.arange(H, dtype=np.float64))
        x = Ki[0, 0] * gx + Ki[0, 1] * gy + Ki[0, 2]
        y = Ki[1, 0] * gx + Ki[1, 1] * gy + Ki[1, 2]
        z = Ki[2, 0] * gx + Ki[2, 1] * gy + Ki[2, 2]
        uv.append((np.float32(x / z), np.float32(y / z)))

    cA = 1.0 / (dmin - dmax)
    cB = -dmax / (dmin - dmax)

    in_maps = []
    for c in range(NCORES):
        b, rc = c // 4, c % 4
        r0 = rc * RPC
        rtop = max(r0 - 1, 0)
        rbot = min(r0 + RPC, H - 1)

        consts = np.zeros((128, NCONST), np.float32)
        for nn in range(NN):
            Kn = K_nei[nn, b, 0, 0].astype(np.float64)
            Rn = R_nei[nn, b, 0, 0].astype(np.float64)
            Tn = T_nei[nn, b, 0, 0].astype(np.float64).reshape(3)
            M = Kn @ Rn
            t = (Kn @ Tn.reshape(3, 1)).reshape(3)
            iK = np.linalg.inv(Kn)
            assert abs(iK[2, 0]) < 1e-12 and abs(iK[2, 1]) < 1e-12 and abs(iK[2, 2] - 1) < 1e-9
            row = np.zeros(NCONST, np.float32)
            row[C_M00:C_M22 + 1] = M.reshape(-1)
            row[C_T0:C_T2 + 1] = t
            row[C_R00:C_R22 + 1] = Rn.reshape(-1)
            row[C_A0:C_A2 + 1] = iK[0] / (1.0 + EPS)
            row[C_B0:C_B2 + 1] = iK[1] / (1.0 + EPS)
            # C_TX/C_TY feed |tz*n + c| as ACT affine bias -> store negated
            row[C_TX], row[C_TY], row[C_TZ] = -Tn[0], -Tn[1], Tn[2]
            row[C_CA], row[C_CB] = cA, cB
            row[C_TEN] = 10.0
            row[C_DS], row[C_DB] = dmin - dmax, dmax
            consts[nn * 64 : nn * 64 + 64] = row

        u_full, v_full = uv[b]
        d_full = depth0[b, 0]

        pix = np.zeros((128, 4, 330), np.float32)
        for nn in range(NN):
            sl = slice(nn * 64, nn * 64 + 64)
            f_full = flow[nn, b, 0]
            for ch, arr in enumerate((u_full, v_full, d_full, f_full)):
                pix[sl, ch, 0:320] = arr[r0 : r0 + RPC]
                pix[sl, ch, 320:325] = arr[rtop].reshape(64, 5)
                pix[sl, ch, 325:330] = arr[rbot].reshape(64, 5)

        hm = np.ones((128, 10), np.float32)
        if r0 == 0:
            hm[:, 0:5] = 0.0
        if r0 + RPC == H:
            hm[:, 5:10] = 0.0

        confpad = np.zeros((NN, 66, 322), np.float32)
        confpad[:, 1:65, 1:321] = conf[:, b, 0, r0 : r0 + RPC, :]
        if r0 > 0:
            confpad[:, 0, 1:321] = conf[:, b, 0, r0 - 1, :]
        if r0 + RPC < H:
            confpad[:, 65, 1:321] = conf[:, b, 0, r0 + RPC, :]

        # [nn, k, p, q, r, wc, w] -> [p, wc, nn, r, q, w, k]
        ms = mask[:, b, :, r0 : r0 + RPC, :].reshape(NN, 9, 4, 4, RPC, 2, 160)
        mask_pk = np.ascontiguousarray(ms.transpose(2, 5, 0, 4, 3, 6, 1)).reshape(
            4, 2, 128, 4, 160, 9
        )

        in_maps.append(
            {
                "pix": pix,
                "consts": consts,
                "hm": hm,
                "confpad": confpad,
                "maskpk": mask_pk,
            }
        )
    return in_maps


def kernel(**inputs):
    if "nc" not in _cache:
        _cache["nc"] = _build_program()
    nc = _cache["nc"]
    in_maps = _host_prep(inputs)

    from concourse import bass_utils

    res = bass_utils.run_bass_kernel_spmd(nc, in_maps, core_ids=list(range(NCORES)))
    out = np.empty((B, 1, H * UP, W * UP), np.float32)
    for c in range(NCORES):
        b, rc = c // 4, c % 4
        out[b, 0, rc * RPC * UP : (rc + 1) * RPC * UP, :] = res.results[c]["out"]
    return out



# revision 22
# speedup vs baseline: 1.1732x; 1.1732x over previous
"""DispMVS depth-fusion kernel for 8 Trainium2 NeuronCores (v5).

Sharding: core c = (b, rh, wh): batch b = c // 4, row-half rh = (c // 2) % 2
(128 of 256 coarse rows), col-half wh = c % 2 (160 of 320 coarse cols).
Partitions = 128 rows; BOTH neighbor streams (NN=2) live on the free axis, so
the cross-neighbor confidence fusion is partition-local with full 128-lane
ops and no SBUF->SBUF partition moves.

Per-core pipeline (identical Bass/Tile program on all 8 cores):
  1. geometry: epipolar math -> clipped inverse depth inv[p, nn, 166]
     (host pre-bakes the pixel-only linear fields a_j, b_j, r_j; per-nn
     scalars enter as stride-0 broadcast tensors).  DRAM scratch round-trip
     rebuilds inv as 3 vertically shifted rows t3i [128, 3, 164] per nn.
  2. 8 chunks (nn, qy) of mask [128, qx4, w160, k9] bf16: exp on ACT;
     softmax denominator via a bf16 pairwise tree (levels on GpSimd, last
     level + f32 cast on DVE); numerators via f32 MUL_CUMSUM against a
     3x3-window AP view of t3 (no unfold-interleave build needed), with
     every-9th strided-diff subs.
  3. confidence fusion batched at the end: sigmoid weights on ACT (one
     table reload), convex combine, affine + reciprocal -> contiguous
     [512, 640] store per core.
"""

import numpy as np

NN, B, H, W = 2, 2, 256, 320
UP = 4
EPS = 1e-6
NCORES = 8
RP = 128          # coarse rows per core
WC = 160          # coarse cols per core
GX = 166          # geometry cols: 162 (= 160 + 2 col-halo) + 4 packed halo-row px
CPAD = 9          # zero prefix cols in cum tile

# broadcast-consts columns (per nn)
C_T0, C_T1, C_T2, C_SA, C_CA, C_SB, C_CB, C_TXN, C_TYN, C_TZ = range(10)
NCB = 10
# per-partition consts columns
K_CA, K_CB, K_DS, K_DB = 0, 1, 2, 3
NCONST = 4

_cache = {}


def _register_custom_ops():
    """Register custom DVE ops (idempotent)."""
    from concourse import dve_ops
    from concourse.dve_spec import AluOp, C0, C1, Spec, Src0, Src1, _has_src1, lower, scan
    from concourse.dve_uop import DveOpSpec

    have = {o.name: o for o in dve_ops.OPS}
    if "MUL_CUMSUM_ANT" in have:
        return have

    def cum_ref(in0, in1, s0, s1, imm2):
        a = in0.astype(np.float32).reshape(in0.shape[0], -1) * in1.astype(
            np.float32
        ).reshape(in1.shape[0], -1)
        return np.cumsum(a, axis=1, dtype=np.float32).reshape(in0.shape)

    specs = [
        ("MUL_CUMSUM_ANT", Spec(body=scan(AluOp.ADD, Src0 * Src1), reference=cum_ref)),
        (
            "SUMSQ_ANT",
            Spec(
                body=Src0 * Src0 + Src1 * Src1,
                reference=lambda in0, in1, s0, s1, imm2: (
                    in0.astype(np.float32) ** 2 + in1.astype(np.float32) ** 2
                ),
            ),
        ),
        (
            "RSQRT_NR_ANT",
            Spec(
                body=(Src0 * Src0 * Src1 * C0 + C1) * Src0,
                reference=lambda in0, in1, s0, s1, imm2: (
                    (in0.astype(np.float32) ** 2 * in1 * s0 + s1) * in0
                ),
            ),
        ),
    ]
    out = dict(have)
    for name, spec in specs:
        op = dve_ops.DveOp(name, spec, subdim=False, uops_sha={})
        dve_ops.OPS.append(op)
        dve_ops.CUSTOM_DVE_SPECS[name] = spec
        dve_ops._SUB_OPCODE_FOR_NAME[name] = (
            dve_ops._CUSTOM_DVE_ROW_BASE + len(dve_ops.OPS) - 1
        )
        for ver in ("v3", "v4"):
            tmp = DveOpSpec(
                name=name,
                opcode=dve_ops.get_dve_sub_opcode(name),
                uops=lower(spec, ver=ver),
                rd1_en=_has_src1(spec),
            )
            op.uops_sha[ver] = tmp.sha(ver)
        out[name] = op
    assert max(dve_ops._SUB_OPCODE_FOR_NAME.values()) < 0x20
    return out


def _build_program():
    import concourse.bass as bass
    import concourse.bacc as bacc
    import concourse.tile as tile
    from concourse import mybir
    from concourse.alu_op_type import AluOpType as op

    f32 = mybir.dt.float32
    bf16 = mybir.dt.bfloat16
    f16 = mybir.dt.float16
    i32 = mybir.dt.int32
    Act = mybir.ActivationFunctionType

    cops = _register_custom_ops()
    nc = bacc.Bacc("TRN2", target_bir_lowering=False, debug=False)

    pix_d = nc.dram_tensor("pix", [128, NN, 2, GX], f32, kind="ExternalInput").ap()
    geo_d = nc.dram_tensor("geo", [128, NN, 9, GX], f32, kind="ExternalInput").ap()
    hm_d = nc.dram_tensor("hm", [128, NN, GX], f32, kind="ExternalInput").ap()
    cbc_d = nc.dram_tensor("cbc", [128, NN, NCB], f32, kind="ExternalInput").ap()
    consts_d = nc.dram_tensor("consts", [128, NCONST], f32, kind="ExternalInput").ap()
    confpad_d = nc.dram_tensor("confpad", [NN, 130, 164], f32, kind="ExternalInput").ap()
    mask_d = nc.dram_tensor("maskpk", [NN, 4, 128, 5760], f16, kind="ExternalInput").ap()
    scr = nc.dram_tensor("scr", [NN, 130, 164], f32, kind="Internal").ap()
    out_d = nc.dram_tensor("out", [RP * UP, WC * UP], f32, kind="ExternalOutput").ap()

    def dram_ap(base, off, dims):
        return bass.AP(tensor=base.tensor, offset=base.offset + off, ap=[list(d) for d in dims])

    def sb_ap(t, off, dims):
        a = t[:]
        return bass.AP(tensor=a.tensor, offset=a.offset + off,
                       ap=[list(a.ap[0])] + [list(d) for d in dims])

    with tile.TileContext(nc) as tc:
        with tc.tile_pool(name="persist", bufs=1) as pp:
            # ---------- persistent loads ----------
            pix = pp.tile([128, NN, 2, GX], f32, name="pix")
            geo = pp.tile([128, NN, 9, GX], f32, name="geo")
            hm = pp.tile([128, NN, GX], f32, name="hm")
            cbc = pp.tile([128, NN, NCB], f32, name="cbc")
            consts = pp.tile([128, NCONST], f32, name="consts")
            nc.sync.dma_start(out=pix[:], in_=pix_d)
            nc.sync.dma_start(out=geo[:], in_=geo_d)
            nc.sync.dma_start(out=hm[:], in_=hm_d)
            nc.sync.dma_start(out=cbc[:], in_=cbc_d)
            nc.sync.dma_start(out=consts[:], in_=consts_d)

            t3c = pp.tile([128, NN, 3, 164], f32, name="t3c")
            for nn in range(NN):
                src = dram_ap(confpad_d, nn * 130 * 164,
                              [[164, 128], [164, 3], [1, 164]])
                nc.sync.dma_start(out=t3c[:, nn], in_=src)

            def CB(nn_, i):
                return cbc[:, nn_, i:i + 1].broadcast_to([128, GX])

            d_ch = pix[:, :, 0, :]     # depth (both nn views identical data per nn)
            fl_ch = pix[:, :, 1, :]    # flow

            ep_ctx = tc.tile_pool(name="geom", bufs=1)
            gp = ep_ctx.__enter__()
            _t = [0]

            def T(shape=(128, NN, GX)):
                _t[0] += 1
                return gp.tile(list(shape), f32, name=f"g{_t[0]}", tag=f"g{_t[0]}")

            def VTT(o, a, b, alu):
                nc.vector.tensor_tensor(out=o, in0=a, in1=b, op=alu)

            def GTT(o, a, b, alu):
                nc.vector.tensor_tensor(out=o, in0=a, in1=b, op=alu)

            def TS(o, a, s1, o0, s2=None, o1=None):
                if o1 is None:
                    nc.vector.tensor_scalar(out=o, in0=a, scalar1=s1, scalar2=None, op0=o0)
                else:
                    nc.vector.tensor_scalar(out=o, in0=a, scalar1=s1, scalar2=s2, op0=o0, op1=o1)

            def STT(o, a, s, b, o0, o1):
                nc.vector.scalar_tensor_tensor(out=o, in0=a, scalar=s, in1=b, op0=o0, op1=o1)

            def AB(o, a):
                nc.scalar.activation(out=o, in_=a, func=Act.Abs)

            # ---------------- geometry ----------------
            # m_j = a_j * d ; ps_j = m_j + t_j ; pe_j = 2*m_j + b_j
            m0, m1, m2 = T(), T(), T()
            ps0, ps1, ps2 = T(), T(), T()
            pe0, pe1, pe2 = T(), T(), T()
            for j, (mj, psj, pej) in enumerate(((m0, ps0, pe0), (m1, ps1, pe1), (m2, ps2, pe2))):
                GTT(mj[:], geo[:, :, j, :], d_ch, op.mult)
                for nn in range(NN):
                    VTT(psj[:, nn], mj[:, nn], CB(nn, C_T0 + j), op.add)
                STT(pej[:], mj[:], 2.0, geo[:, :, 3 + j, :], op.mult, op.add)

            rs2, re2 = T(), T()
            tmp = T()
            rscr = T()
            AB(tmp[:], ps2[:])
            TS(tmp[:], tmp[:], EPS, op.add)
            nc.vector.reciprocal_approx_accurate(out=rs2[:], in_=tmp[:], scratch=rscr[:])
            AB(tmp[:], pe2[:])
            TS(tmp[:], tmp[:], EPS, op.add)
            nc.vector.reciprocal_approx_accurate(out=re2[:], in_=tmp[:], scratch=rscr[:])

            pxs, pys, pxe, pye = T(), T(), T(), T()
            GTT(pxs[:], ps0[:], rs2[:], op.mult)
            GTT(pys[:], ps1[:], rs2[:], op.mult)
            GTT(pxe[:], pe0[:], re2[:], op.mult)
            GTT(pye[:], pe1[:], re2[:], op.mult)

            fdx, fdy = T(), T()
            VTT(fdx[:], pxe[:], pxs[:], op.subtract)
            VTT(fdy[:], pye[:], pys[:], op.subtract)

            # rsqrt(fdx^2+fdy^2): magic seed + 2 Newton steps
            q = T()
            nc.vector._custom_dve(cops["SUMSQ_ANT"], out=q[:], in0=fdx[:], in1=fdy[:])
            y = T()
            yi = y[:].bitcast(i32)
            TS(yi, q[:].bitcast(i32), 1, op.arith_shift_right)
            TS(yi, yi, -1, op.bitwise_xor)
            TS(yi, yi, 0x5F3759DF + 1, op.add)
            y2 = T()
            nc.vector._custom_dve(cops["RSQRT_NR_ANT"], out=y2[:], in0=y[:], in1=q[:], s0=-0.5, s1=1.5)
            nc.vector._custom_dve(cops["RSQRT_NR_ANT"], out=y[:], in0=y2[:], in1=q[:], s0=-0.5, s1=1.5)

            fls = T()
            GTT(fls[:], fl_ch, y[:], op.mult)
            mx, my = T(), T()
            GTT(tmp[:], fdx[:], fls[:], op.mult)
            VTT(mx[:], tmp[:], pxs[:], op.add)
            GTT(tmp[:], fdy[:], fls[:], op.mult)
            VTT(my[:], tmp[:], pys[:], op.add)

            fm = T()
            fmi = fm[:].bitcast(i32)
            ax = T()
            AB(ax[:], fdx[:])
            AB(tmp[:], fdy[:])
            VTT(fmi, tmp[:], ax[:], op.is_gt)

            # n = iK @ [mx, my, 1]: nx = mx*sA + cA ; ny = my*sB + cB  (per nn)
            nx, ny = T(), T()
            for nn in range(NN):
                GTT(tmp[:, nn], mx[:, nn], CB(nn, C_SA), op.mult)
                VTT(nx[:, nn], tmp[:, nn], CB(nn, C_CA), op.add)
                GTT(tmp[:, nn], my[:, nn], CB(nn, C_SB), op.mult)
                VTT(ny[:, nn], tmp[:, nn], CB(nn, C_CB), op.add)

            # select axis first, then one triangulation
            rsel, nsel, tsel = T(), T(), T()
            for nn in range(NN):
                fmn = fm[:, nn].bitcast(i32)
                nc.vector.select(out=rsel[:, nn], mask=fmn, on_true=geo[:, nn, 7, :], on_false=geo[:, nn, 6, :])
                nc.vector.select(out=nsel[:, nn], mask=fmn, on_true=ny[:, nn], on_false=nx[:, nn])
            txb, tyb = T(), T()
            for nn in range(NN):
                nc.scalar.activation(out=txb[:, nn], in_=CB(nn, C_TXN), func=Act.Copy)
                nc.scalar.activation(out=tyb[:, nn], in_=CB(nn, C_TYN), func=Act.Copy)
            for nn in range(NN):
                nc.vector.select(out=tsel[:, nn], mask=fm[:, nn].bitcast(i32), on_true=tyb[:, nn], on_false=txb[:, nn])

            num = T()
            GTT(tmp[:], geo[:, :, 8, :], nsel[:], op.mult)   # rz * nsel
            VTT(tmp[:], rsel[:], tmp[:], op.subtract)
            AB(num[:], tmp[:])
            dn = T()
            for nn in range(NN):
                GTT(dn[:, nn], nsel[:, nn], CB(nn, C_TZ), op.mult)
            VTT(dn[:], dn[:], tsel[:], op.add)               # tz*n + (-t)
            AB(dn[:], dn[:])
            TS(dn[:], dn[:], EPS, op.add)
            rdn = T()
            nc.vector.reciprocal_approx_accurate(out=rdn[:], in_=dn[:], scratch=rscr[:])
            inv = T()
            GTT(inv[:], num[:], rdn[:], op.mult)

            # clip to [0,1] after affine (dmax..dmin normalize), zero invalid px
            invc = T()
            nc.scalar.activation(out=invc[:], in_=inv[:], func=Act.Identity,
                                 scale=consts[:, K_CA:K_CA + 1], bias=consts[:, K_CB:K_CB + 1])
            TS(invc[:], invc[:], 0.0, op.max, 1.0, op.min)
            VTT(invc[:], invc[:], hm[:], op.mult)

            # ------- scratch round-trip -------
            for nn in range(NN):
                base = nn * 130 * 164
                nc.sync.dma_start(
                    out=dram_ap(scr, base + 164, [[164, 128], [1, 162]]),
                    in_=invc[:, nn, 0:162],
                )
                # packed halo rows: cols 162-163 -> scr row 0; 164-165 -> row 129
                nc.sync.dma_start(
                    out=dram_ap(scr, base, [[2, 81], [1, 2]]),
                    in_=invc[0:81, nn, 162:164],
                )
                nc.sync.dma_start(
                    out=dram_ap(scr, base + 129 * 164, [[2, 81], [1, 2]]),
                    in_=invc[0:81, nn, 164:166],
                )
                # zero pad cols 162..163 of rows 0..129 (never read, but keep clean)
            t3i = pp.tile([128, NN, 3, 164], f32, name="t3i")
            for nn in range(NN):
                src = dram_ap(scr, nn * 130 * 164, [[164, 128], [164, 3], [1, 164]])
                nc.sync.dma_start(out=t3i[:, nn], in_=src)

            ep_ctx.__exit__(None, None, None)

            # ---------------- chunk loop ----------------
            iu = pp.tile([128, NN, 4, 640], f32, name="iu")   # per (nn, qy)
            cu = pp.tile([128, NN, 4, 640], f32, name="cu")

            # interleaved unfold weights uf9[p, nn, w, k] (k = dy*3+dx innermost)
            uf9i = pp.tile([128, NN, WC, 9], f32, name="uf9i")
            uf9c = pp.tile([128, NN, WC, 9], f32, name="uf9c")
            for t3, uf9 in ((t3c, uf9c), (t3i, uf9i)):
                for dy in range(3):
                    for dx in range(3):
                        nc.vector.tensor_scalar(
                            out=uf9[:, :, :, dy * 3 + dx],
                            in0=t3[:, :, dy, dx:dx + WC],
                            scalar1=1.0, scalar2=None, op0=op.mult)

            with tc.tile_pool(name="chunk", bufs=2) as cp, tc.tile_pool(
                name="chunk1", bufs=1
            ) as cp1:
                for qy in range(4):
                    for nn in range(NN):
                        e = cp.tile([128, 5760], f16, name="e", tag="e")
                        nc.sync.dma_start(
                            out=e[:],
                            in_=dram_ap(mask_d, (nn * 4 + qy) * 128 * 5760,
                                        [[5760, 128], [1, 5760]]))
                        nc.scalar.activation(out=e[:], in_=e[:], func=Act.Exp)

                        # --- den: grouped reduce over k (fp16 in, f32 out) ---
                        den = cp1.tile([128, 640], f32, name="den", tag="den")
                        nc.vector.tensor_reduce(
                            out=den[:], in_=e[:].rearrange("p (g k) -> p g k", k=9),
                            axis=mybir.AxisListType.X, op=op.add)
                        rs = cp1.tile([128, 640], f32, name="rs", tag="rs")
                        nc.vector.reciprocal_approx_fast(out=rs[:], in_=den[:])

                        # --- numerators: cumsum + strided diff ---
                        cum = cp1.tile([128, 4, CPAD + 1440], f32, name="cum", tag="cum")
                        nc.vector.memset(sb_ap(cum, 0, [[CPAD + 1440, 4], [1, CPAD]]), 0.0)
                        for tag, uf9 in (("c", uf9c), ("i", uf9i)):
                            for qx in range(4):
                                nc.vector._custom_dve(
                                    cops["MUL_CUMSUM_ANT"],
                                    out=sb_ap(cum, qx * (CPAD + 1440) + CPAD, [[1, 1440]]),
                                    in0=sb_ap(e, qx * 1440, [[1, 1440]]),
                                    in1=uf9[:, nn].rearrange("p a b -> p (a b)"))
                            acc = cp1.tile([128, 640], f32, name="acc", tag="acc" + tag)
                            nc.vector.tensor_tensor(
                                out=acc[:],
                                in0=sb_ap(cum, CPAD + 8, [[CPAD + 1440, 4], [9, WC]]),
                                in1=sb_ap(cum, CPAD - 1, [[CPAD + 1440, 4], [9, WC]]),
                                op=op.subtract)
                            dst = cu if tag == "c" else iu
                            nc.vector.tensor_tensor(out=dst[:, nn, qy], in0=acc[:], in1=rs[:], op=op.mult)

                # ---------------- fusion (batched) ----------------
                for qy in range(4):
                    dif = cp1.tile([128, 640], f32, name="dif", tag="dif")
                    nc.vector.tensor_tensor(out=dif[:], in0=cu[:, 1, qy], in1=cu[:, 0, qy], op=op.subtract)
                    dm = cp1.tile([128, 640], f32, name="dm", tag="dm")
                    nc.vector.tensor_tensor(out=dm[:], in0=iu[:, 1, qy], in1=iu[:, 0, qy], op=op.subtract)
                    s1 = cp1.tile([128, 640], f32, name="s1", tag="s1")
                    nc.scalar.activation(out=s1[:], in_=dif[:], func=Act.Sigmoid)
                    t = cp1.tile([128, 640], f32, name="t", tag="t")
                    nc.vector.tensor_tensor(out=t[:], in0=s1[:], in1=dm[:], op=op.mult)
                    fus = cp1.tile([128, 640], f32, name="fus", tag="fus")
                    nc.vector.tensor_tensor(out=fus[:], in0=t[:], in1=iu[:, 0, qy], op=op.add)
                    aff = cp1.tile([128, 640], f32, name="aff", tag="aff")
                    nc.scalar.activation(out=aff[:], in_=fus[:], func=Act.Identity,
                                         scale=consts[:, K_DS:K_DS + 1], bias=consts[:, K_DB:K_DB + 1])
                    ot = cp1.tile([128, 160, 4], f32, name="ot", tag="ot")
                    nc.vector.reciprocal_approx_fast(
                        out=ot[:].rearrange("p w q -> p q w"),
                        in_=aff[:].rearrange("p (q w) -> p q w", q=4))
                    nc.sync.dma_start(
                        out=dram_ap(out_d, qy * WC * UP, [[UP * WC * UP, 128], [1, 640]]),
                        in_=ot[:].rearrange("p a b -> p (a b)"))

    nc.finalize()
    return nc


def _host_prep(inputs):
    K_ref = np.asarray(inputs["K_ref"], np.float32)
    K_nei = np.asarray(inputs["K_nei"], np.float32)
    R_nei = np.asarray(inputs["R_nei"], np.float32)
    T_nei = np.asarray(inputs["T_nei"], np.float32)
    depth0 = np.asarray(inputs["depth0"], np.float32)
    flow = np.asarray(inputs["flow"], np.float32)
    mask = np.asarray(inputs["mask"], np.float32)
    conf = np.asarray(inputs["conf"], np.float32)
    dmin = float(np.asarray(inputs["depth_min"]).reshape(-1)[0])
    dmax = float(np.asarray(inputs["depth_max"]).reshape(-1)[0])

    cA = 1.0 / (dmin - dmax)
    cB = -dmax / (dmin - dmax)

    # pixel rays per batch (unit-z), float64 on host
    uv = []
    for b in range(B):
        Ki = np.linalg.inv(K_ref[b, 0, 0].astype(np.float64))
        gx, gy = np.meshgrid(np.arange(W, dtype=np.float64), np.arange(H, dtype=np.float64))
        x = Ki[0, 0] * gx + Ki[0, 1] * gy + Ki[0, 2]
        yy = Ki[1, 0] * gx + Ki[1, 1] * gy + Ki[1, 2]
        z = Ki[2, 0] * gx + Ki[2, 1] * gy + Ki[2, 2]
        uv.append((x / z, yy / z))

    in_maps = []
    for c in range(NCORES):
        b, rh, wh = c // 4, (c // 2) % 2, c % 2
        r0, w0 = rh * RP, wh * WC

        # geometry pixel grid: rows r0..r0+127, cols w0-1..w0+160 (162) +
        # packed halo rows (r0-1, r0+128) x 162 -> cols 162..165
        rows = np.arange(r0, r0 + RP)
        cols = np.clip(np.arange(w0 - 1, w0 + WC + 1), 0, W - 1)  # 162, edge-clamped
        rtop = max(r0 - 1, 0)
        rbot = min(r0 + RP, H - 1)

        pix = np.zeros((128, NN, 2, GX), np.float32)
        geo = np.zeros((128, NN, 9, GX), np.float32)
        hm = np.ones((128, NN, GX), np.float32)
        cbc = np.zeros((128, NN, NCB), np.float32)
        consts = np.zeros((128, NCONST), np.float32)
        consts[:, K_CA] = cA
        consts[:, K_CB] = cB
        consts[:, K_DS] = dmin - dmax
        consts[:, K_DB] = dmax

        ug, vg = uv[b]

        def gather(arr):
            """arr [H, W] -> [128, GX]: main 162 cols + packed halo rows."""
            out = np.zeros((128, GX), np.float32)
            out[:, 0:162] = arr[np.ix_(rows, cols)]
            halo_t = arr[rtop][cols]            # 162
            halo_b = arr[rbot][cols]
            out[0:81, 162:164] = halo_t.reshape(81, 2)
            out[0:81, 164:166] = halo_b.reshape(81, 2)
            return out

        u_g = gather(ug)
        v_g = gather(vg)
        d_g = gather(depth0[b, 0].astype(np.float64))
        for nn in range(NN):
            fl_g = gather(flow[nn, b, 0].astype(np.float64))
            pix[:, nn, 0] = d_g
            pix[:, nn, 1] = fl_g

            Kn = K_nei[nn, b, 0, 0].astype(np.float64)
            Rn = R_nei[nn, b, 0, 0].astype(np.float64)
            Tn = T_nei[nn, b, 0, 0].astype(np.float64).reshape(3)
            M = Kn @ Rn
            t = (Kn @ Tn.reshape(3, 1)).reshape(3)
            iK = np.linalg.inv(Kn)
            assert abs(iK[0, 1]) < 1e-12 and abs(iK[1, 0]) < 1e-12
            assert abs(iK[2, 0]) < 1e-12 and abs(iK[2, 1]) < 1e-12 and abs(iK[2, 2] - 1) < 1e-9

            for j in range(3):
                a_j = M[j, 0] * u_g + M[j, 1] * v_g + M[j, 2]
                geo[:, nn, j] = a_j
                geo[:, nn, 3 + j] = 10.0 * a_j + t[j]
            for j in range(3):
                geo[:, nn, 6 + j] = Rn[j, 0] * u_g + Rn[j, 1] * v_g + Rn[j, 2]

            cbc[:, nn, C_T0] = t[0]
            cbc[:, nn, C_T1] = t[1]
            cbc[:, nn, C_T2] = t[2]
            s = 1.0 + EPS
            cbc[:, nn, C_SA] = iK[0, 0] / s
            cbc[:, nn, C_CA] = iK[0, 2] / s
            cbc[:, nn, C_SB] = iK[1, 1] / s
            cbc[:, nn, C_CB] = iK[1, 2] / s
            cbc[:, nn, C_TXN] = -Tn[0]
            cbc[:, nn, C_TYN] = -Tn[1]
            cbc[:, nn, C_TZ] = Tn[2]

        # hm zeros: invalid halo cols / rows
        if w0 == 0:
            hm[:, :, 0] = 0.0
            hm[0, :, 162] = 0.0   # packed halo rows, left-edge px
            hm[0, :, 164] = 0.0
        if w0 + WC == W:
            hm[:, :, 161] = 0.0
            hm[80, :, 163] = 0.0  # packed halo rows, right-edge px
            hm[80, :, 165] = 0.0
        if r0 == 0:
            hm[:, :, 162:164] = 0.0
        if r0 + RP == H:
            hm[:, :, 164:166] = 0.0
        hm[81:, :, 162:166] = 0.0  # unused packed slots

        confpad = np.zeros((NN, 130, 164), np.float32)
        cw = np.arange(w0 - 1, w0 + WC + 1)
        cwv = (cw >= 0) & (cw < W)
        confpad[:, 1:129, 0:162][:, :, cwv] = conf[:, b, 0, r0:r0 + RP][:, :, cw[cwv]]
        if r0 > 0:
            confpad[:, 0, 0:162][:, cwv] = conf[:, b, 0, r0 - 1][:, cw[cwv]]
        if r0 + RP < H:
            confpad[:, 129, 0:162][:, cwv] = conf[:, b, 0, r0 + RP][:, cw[cwv]]

        # mask: [nn, qy, p, (qx, w, k)] with k = dy*3+dx row-major
        ms = mask[:, b, :, r0:r0 + RP, w0:w0 + WC]          # [NN, 144, 128, 160]
        ms = ms.reshape(NN, 9, 4, 4, RP, WC)               # [NN, k, qy, qx, p, w]
        mask_pk = np.ascontiguousarray(
            ms.transpose(0, 2, 4, 3, 5, 1)                 # [NN, qy, p, qx, w, k]
        ).reshape(NN, 4, 128, 5760).astype(np.float16)

        in_maps.append({
            "pix": pix, "geo": geo, "hm": hm, "cbc": cbc, "consts": consts,
            "confpad": confpad, "maskpk": mask_pk,
        })
    return in_maps


def kernel(**inputs):
    if "nc" not in _cache:
        _cache["nc"] = _build_program()
    nc = _cache["nc"]
    in_maps = _host_prep(inputs)

    from concourse import bass_utils

    res = bass_utils.run_bass_kernel_spmd(nc, in_maps, core_ids=list(range(NCORES)))
    out = np.empty((B, 1, H * UP, W * UP), np.float32)
    for c in range(NCORES):
        b, rh, wh = c // 4, (c // 2) % 2, c % 2
        out[b, 0, rh * RP * UP:(rh + 1) * RP * UP, wh * WC * UP:(wh + 1) * WC * UP] = res.results[c]["out"]
    return out


# revision 25
# speedup vs baseline: 1.2459x; 1.0619x over previous
"""DispMVS depth-fusion kernel for 8 Trainium2 NeuronCores (v5).

Sharding: core c = (b, rh, wh): batch b = c // 4, row-half rh = (c // 2) % 2
(128 of 256 coarse rows), col-half wh = c % 2 (160 of 320 coarse cols).
Partitions = 128 rows; BOTH neighbor streams (NN=2) live on the free axis, so
the cross-neighbor confidence fusion is partition-local with full 128-lane
ops and no SBUF->SBUF partition moves.

Per-core pipeline (identical Bass/Tile program on all 8 cores):
  1. geometry: epipolar math -> clipped inverse depth inv[p, nn, 166]
     (host pre-bakes the pixel-only linear fields a_j, b_j, r_j; per-nn
     scalars enter as stride-0 broadcast tensors).  DRAM scratch round-trip
     rebuilds inv as 3 vertically shifted rows t3i [128, 3, 164] per nn.
  2. 8 chunks (nn, qy) of mask [128, qx4, w160, k9] fp16: exp in-place on
     ACT; softmax denominator via grouped tensor_reduce (fp16 in, f32 out);
     numerators via f32 MUL_CUMSUM scans (fp16 e) against interleaved
     unfold-weight tiles, extracting group sums with every-9th strided-diff
     subtracts against a zero-padded prefix.  GpSimd is deliberately idle:
     concurrent GpSimd SBUF traffic degrades DVE throughput ~4x.
  3. confidence fusion batched at the end: sigmoid weights on ACT (one
     table reload), convex combine, affine + reciprocal -> contiguous
     [512, 640] store per core.
"""

import numpy as np

NN, B, H, W = 2, 2, 256, 320
UP = 4
EPS = 1e-6
NCORES = 8
RP = 128          # coarse rows per core
WC = 160          # coarse cols per core
GX = 166          # geometry cols: 162 (= 160 + 2 col-halo) + 4 packed halo-row px
CPAD = 9          # zero prefix cols in cum tile

# broadcast-consts columns (per nn)
C_T0, C_T1, C_T2, C_SA, C_CA, C_SB, C_CB, C_TXN, C_TYN, C_TZ = range(10)
NCB = 10
# per-partition consts columns
K_CA, K_CB, K_DS, K_DB = 0, 1, 2, 3
NCONST = 4

_cache = {}


def _register_custom_ops():
    """Register custom DVE ops (idempotent)."""
    from concourse import dve_ops
    from concourse.dve_spec import AluOp, C0, C1, Spec, Src0, Src1, _has_src1, lower, scan
    from concourse.dve_uop import DveOpSpec

    have = {o.name: o for o in dve_ops.OPS}
    if "MUL_CUMSUM_ANT" in have:
        return have

    def cum_ref(in0, in1, s0, s1, imm2):
        a = in0.astype(np.float32).reshape(in0.shape[0], -1) * in1.astype(
            np.float32
        ).reshape(in1.shape[0], -1)
        return np.cumsum(a, axis=1, dtype=np.float32).reshape(in0.shape)

    specs = [
        ("MUL_CUMSUM_ANT", Spec(body=scan(AluOp.ADD, Src0 * Src1), reference=cum_ref)),
        (
            "SUMSQ_ANT",
            Spec(
                body=Src0 * Src0 + Src1 * Src1,
                reference=lambda in0, in1, s0, s1, imm2: (
                    in0.astype(np.float32) ** 2 + in1.astype(np.float32) ** 2
                ),
            ),
        ),
        (
            "RSQRT_NR_ANT",
            Spec(
                body=(Src0 * Src0 * Src1 * C0 + C1) * Src0,
                reference=lambda in0, in1, s0, s1, imm2: (
                    (in0.astype(np.float32) ** 2 * in1 * s0 + s1) * in0
                ),
            ),
        ),
    ]
    out = dict(have)
    for name, spec in specs:
        op = dve_ops.DveOp(name, spec, subdim=False, uops_sha={})
        dve_ops.OPS.append(op)
        dve_ops.CUSTOM_DVE_SPECS[name] = spec
        dve_ops._SUB_OPCODE_FOR_NAME[name] = (
            dve_ops._CUSTOM_DVE_ROW_BASE + len(dve_ops.OPS) - 1
        )
        for ver in ("v3", "v4"):
            tmp = DveOpSpec(
                name=name,
                opcode=dve_ops.get_dve_sub_opcode(name),
                uops=lower(spec, ver=ver),
                rd1_en=_has_src1(spec),
            )
            op.uops_sha[ver] = tmp.sha(ver)
        out[name] = op
    assert max(dve_ops._SUB_OPCODE_FOR_NAME.values()) < 0x20
    return out


def _build_program():
    import concourse.bass as bass
    import concourse.bacc as bacc
    import concourse.tile as tile
    from concourse import mybir
    from concourse.alu_op_type import AluOpType as op

    f32 = mybir.dt.float32
    bf16 = mybir.dt.bfloat16
    f16 = mybir.dt.float16
    i32 = mybir.dt.int32
    Act = mybir.ActivationFunctionType

    cops = _register_custom_ops()
    nc = bacc.Bacc("TRN2", target_bir_lowering=False, debug=False)

    pix_d = nc.dram_tensor("pix", [128, NN, 2, GX], f32, kind="ExternalInput").ap()
    geo_d = nc.dram_tensor("geo", [128, NN, 9, GX], f32, kind="ExternalInput").ap()
    hm_d = nc.dram_tensor("hm", [128, NN, GX], f32, kind="ExternalInput").ap()
    cbc_d = nc.dram_tensor("cbc", [128, NN, NCB], f32, kind="ExternalInput").ap()
    consts_d = nc.dram_tensor("consts", [128, NCONST], f32, kind="ExternalInput").ap()
    confpad_d = nc.dram_tensor("confpad", [NN, 130, 164], f32, kind="ExternalInput").ap()
    mask_d = nc.dram_tensor("maskpk", [NN, 4, 128, 5760], f16, kind="ExternalInput").ap()
    masko_d = nc.dram_tensor("maskpo", [NN, 4, 128, 9, 640], f16, kind="ExternalInput").ap()
    scr = nc.dram_tensor("scr", [NN, 130, 164], f32, kind="Internal").ap()
    out_d = nc.dram_tensor("out", [RP * UP, WC * UP], f32, kind="ExternalOutput").ap()

    def dram_ap(base, off, dims):
        return bass.AP(tensor=base.tensor, offset=base.offset + off, ap=[list(d) for d in dims])

    def sb_ap(t, off, dims):
        a = t[:]
        return bass.AP(tensor=a.tensor, offset=a.offset + off,
                       ap=[list(a.ap[0])] + [list(d) for d in dims])

    with tile.TileContext(nc) as tc:
        with tc.tile_pool(name="persist", bufs=1) as pp:
            # ---------- persistent loads ----------
            pix = pp.tile([128, NN, 2, GX], f32, name="pix")
            geo = pp.tile([128, NN, 9, GX], f32, name="geo")
            hm = pp.tile([128, NN, GX], f32, name="hm")
            cbc = pp.tile([128, NN, NCB], f32, name="cbc")
            consts = pp.tile([128, NCONST], f32, name="consts")
            nc.sync.dma_start(out=pix[:], in_=pix_d)
            nc.sync.dma_start(out=geo[:], in_=geo_d)
            nc.sync.dma_start(out=hm[:], in_=hm_d)
            nc.sync.dma_start(out=cbc[:], in_=cbc_d)
            nc.sync.dma_start(out=consts[:], in_=consts_d)

            t3c = pp.tile([128, NN, 3, 164], f32, name="t3c")
            for nn in range(NN):
                src = dram_ap(confpad_d, nn * 130 * 164,
                              [[164, 128], [164, 3], [1, 164]])
                nc.sync.dma_start(out=t3c[:, nn], in_=src)

            def CB(nn_, i):
                return cbc[:, nn_, i:i + 1].broadcast_to([128, GX])

            d_ch = pix[:, :, 0, :]     # depth (both nn views identical data per nn)
            fl_ch = pix[:, :, 1, :]    # flow

            ep_ctx = tc.tile_pool(name="geom", bufs=1)
            gp = ep_ctx.__enter__()
            _t = [0]

            def T(shape=(128, NN, GX)):
                _t[0] += 1
                return gp.tile(list(shape), f32, name=f"g{_t[0]}", tag=f"g{_t[0]}")

            def VTT(o, a, b, alu):
                nc.vector.tensor_tensor(out=o, in0=a, in1=b, op=alu)

            def GTT(o, a, b, alu):
                nc.vector.tensor_tensor(out=o, in0=a, in1=b, op=alu)

            def TS(o, a, s1, o0, s2=None, o1=None):
                if o1 is None:
                    nc.vector.tensor_scalar(out=o, in0=a, scalar1=s1, scalar2=None, op0=o0)
                else:
                    nc.vector.tensor_scalar(out=o, in0=a, scalar1=s1, scalar2=s2, op0=o0, op1=o1)

            def STT(o, a, s, b, o0, o1):
                nc.vector.scalar_tensor_tensor(out=o, in0=a, scalar=s, in1=b, op0=o0, op1=o1)

            def AB(o, a):
                nc.scalar.activation(out=o, in_=a, func=Act.Abs)

            # ---------------- geometry ----------------
            # m_j = a_j * d ; ps_j = m_j + t_j ; pe_j = 2*m_j + b_j
            m0, m1, m2 = T(), T(), T()
            ps0, ps1, ps2 = T(), T(), T()
            pe0, pe1, pe2 = T(), T(), T()
            for j, (mj, psj, pej) in enumerate(((m0, ps0, pe0), (m1, ps1, pe1), (m2, ps2, pe2))):
                GTT(mj[:], geo[:, :, j, :], d_ch, op.mult)
                for nn in range(NN):
                    VTT(psj[:, nn], mj[:, nn], CB(nn, C_T0 + j), op.add)
                STT(pej[:], mj[:], 2.0, geo[:, :, 3 + j, :], op.mult, op.add)

            rs2, re2 = T(), T()
            tmp = T()
            rscr = T()
            AB(tmp[:], ps2[:])
            TS(tmp[:], tmp[:], EPS, op.add)
            nc.vector.reciprocal_approx_accurate(out=rs2[:], in_=tmp[:], scratch=rscr[:])
            AB(tmp[:], pe2[:])
            TS(tmp[:], tmp[:], EPS, op.add)
            nc.vector.reciprocal_approx_accurate(out=re2[:], in_=tmp[:], scratch=rscr[:])

            pxs, pys, pxe, pye = T(), T(), T(), T()
            GTT(pxs[:], ps0[:], rs2[:], op.mult)
            GTT(pys[:], ps1[:], rs2[:], op.mult)
            GTT(pxe[:], pe0[:], re2[:], op.mult)
            GTT(pye[:], pe1[:], re2[:], op.mult)

            fdx, fdy = T(), T()
            VTT(fdx[:], pxe[:], pxs[:], op.subtract)
            VTT(fdy[:], pye[:], pys[:], op.subtract)

            # rsqrt(fdx^2+fdy^2): magic seed + 2 Newton steps
            q = T()
            nc.vector._custom_dve(cops["SUMSQ_ANT"], out=q[:], in0=fdx[:], in1=fdy[:])
            y = T()
            yi = y[:].bitcast(i32)
            TS(yi, q[:].bitcast(i32), 1, op.arith_shift_right)
            TS(yi, yi, -1, op.bitwise_xor)
            TS(yi, yi, 0x5F3759DF + 1, op.add)
            y2 = T()
            nc.vector._custom_dve(cops["RSQRT_NR_ANT"], out=y2[:], in0=y[:], in1=q[:], s0=-0.5, s1=1.5)
            nc.vector._custom_dve(cops["RSQRT_NR_ANT"], out=y[:], in0=y2[:], in1=q[:], s0=-0.5, s1=1.5)

            fls = T()
            GTT(fls[:], fl_ch, y[:], op.mult)
            mx, my = T(), T()
            GTT(tmp[:], fdx[:], fls[:], op.mult)
            VTT(mx[:], tmp[:], pxs[:], op.add)
            GTT(tmp[:], fdy[:], fls[:], op.mult)
            VTT(my[:], tmp[:], pys[:], op.add)

            fm = T()
            fmi = fm[:].bitcast(i32)
            ax = T()
            AB(ax[:], fdx[:])
            AB(tmp[:], fdy[:])
            VTT(fmi, tmp[:], ax[:], op.is_gt)

            # n = iK @ [mx, my, 1]: nx = mx*sA + cA ; ny = my*sB + cB  (per nn)
            nx, ny = T(), T()
            for nn in range(NN):
                GTT(tmp[:, nn], mx[:, nn], CB(nn, C_SA), op.mult)
                VTT(nx[:, nn], tmp[:, nn], CB(nn, C_CA), op.add)
                GTT(tmp[:, nn], my[:, nn], CB(nn, C_SB), op.mult)
                VTT(ny[:, nn], tmp[:, nn], CB(nn, C_CB), op.add)

            # select axis first, then one triangulation
            rsel, nsel, tsel = T(), T(), T()
            for nn in range(NN):
                fmn = fm[:, nn].bitcast(i32)
                nc.vector.select(out=rsel[:, nn], mask=fmn, on_true=geo[:, nn, 7, :], on_false=geo[:, nn, 6, :])
                nc.vector.select(out=nsel[:, nn], mask=fmn, on_true=ny[:, nn], on_false=nx[:, nn])
            txb, tyb = T(), T()
            for nn in range(NN):
                nc.scalar.activation(out=txb[:, nn], in_=CB(nn, C_TXN), func=Act.Copy)
                nc.scalar.activation(out=tyb[:, nn], in_=CB(nn, C_TYN), func=Act.Copy)
            for nn in range(NN):
                nc.vector.select(out=tsel[:, nn], mask=fm[:, nn].bitcast(i32), on_true=tyb[:, nn], on_false=txb[:, nn])

            num = T()
            GTT(tmp[:], geo[:, :, 8, :], nsel[:], op.mult)   # rz * nsel
            VTT(tmp[:], rsel[:], tmp[:], op.subtract)
            AB(num[:], tmp[:])
            dn = T()
            for nn in range(NN):
                GTT(dn[:, nn], nsel[:, nn], CB(nn, C_TZ), op.mult)
            VTT(dn[:], dn[:], tsel[:], op.add)               # tz*n + (-t)
            AB(dn[:], dn[:])
            TS(dn[:], dn[:], EPS, op.add)
            rdn = T()
            nc.vector.reciprocal_approx_accurate(out=rdn[:], in_=dn[:], scratch=rscr[:])
            inv = T()
            GTT(inv[:], num[:], rdn[:], op.mult)

            # clip to [0,1] after affine (dmax..dmin normalize), zero invalid px
            invc = T()
            nc.scalar.activation(out=invc[:], in_=inv[:], func=Act.Identity,
                                 scale=consts[:, K_CA:K_CA + 1], bias=consts[:, K_CB:K_CB + 1])
            TS(invc[:], invc[:], 0.0, op.max, 1.0, op.min)
            VTT(invc[:], invc[:], hm[:], op.mult)

            # ------- scratch round-trip -------
            for nn in range(NN):
                base = nn * 130 * 164
                nc.sync.dma_start(
                    out=dram_ap(scr, base + 164, [[164, 128], [1, 162]]),
                    in_=invc[:, nn, 0:162],
                )
                # packed halo rows: cols 162-163 -> scr row 0; 164-165 -> row 129
                nc.sync.dma_start(
                    out=dram_ap(scr, base, [[2, 81], [1, 2]]),
                    in_=invc[0:81, nn, 162:164],
                )
                nc.sync.dma_start(
                    out=dram_ap(scr, base + 129 * 164, [[2, 81], [1, 2]]),
                    in_=invc[0:81, nn, 164:166],
                )
                # zero pad cols 162..163 of rows 0..129 (never read, but keep clean)
            t3i = pp.tile([128, NN, 3, 164], f32, name="t3i")
            for nn in range(NN):
                src = dram_ap(scr, nn * 130 * 164, [[164, 128], [164, 3], [1, 164]])
                nc.sync.dma_start(out=t3i[:, nn], in_=src)

            ep_ctx.__exit__(None, None, None)

            # ---------------- chunk loop ----------------
            iu = pp.tile([128, NN, 4, 640], f32, name="iu")   # per (nn, qy)
            cu = pp.tile([128, NN, 4, 640], f32, name="cu")

            # interleaved unfold weights uf9[p, nn, w, k] (k = dy*3+dx innermost)
            uf9i = pp.tile([128, NN, WC, 9], f32, name="uf9i")
            uf9c = pp.tile([128, NN, WC, 9], f32, name="uf9c")
            for t3, uf9 in ((t3c, uf9c), (t3i, uf9i)):
                for dy in range(3):
                    for dx in range(3):
                        nc.vector.tensor_scalar(
                            out=uf9[:, :, :, dy * 3 + dx],
                            in0=t3[:, :, dy, dx:dx + WC],
                            scalar1=1.0, scalar2=None, op0=op.mult)

            with tc.tile_pool(name="chunk", bufs=2) as cp, tc.tile_pool(
                name="chunk1", bufs=1
            ) as cp1:
                for qy in range(4):
                    for nn in range(NN):
                        e = cp.tile([128, 5760], f16, name="e", tag="e")
                        nc.sync.dma_start(
                            out=e[:],
                            in_=dram_ap(mask_d, (nn * 4 + qy) * 128 * 5760,
                                        [[5760, 128], [1, 5760]]))
                        nc.scalar.activation(out=e[:], in_=e[:], func=Act.Exp)

                        # --- den: k-outer exp (fp16 in -> bf16) + bf16 DVE tree ---
                        m2 = cp.tile([128, 5760], f16, name="m2", tag="m2")
                        nc.sync.dma_start(
                            out=m2[:],
                            in_=dram_ap(masko_d, (nn * 4 + qy) * 128 * 5760,
                                        [[5760, 128], [1, 5760]]))
                        e2v = m2[:].bitcast(bf16)
                        nc.scalar.activation(out=e2v, in_=m2[:], func=Act.Exp)
                        t4b = cp1.tile([128, 2560], bf16, name="t4b", tag="t4b")
                        nc.vector.tensor_tensor(out=t4b[:], in0=e2v[:, 0:2560], in1=e2v[:, 2560:5120], op=op.add)
                        t2b = cp1.tile([128, 1280], bf16, name="t2b", tag="t2b")
                        nc.vector.tensor_tensor(out=t2b[:], in0=t4b[:, 0:1280], in1=t4b[:, 1280:2560], op=op.add)
                        dh = cp1.tile([128, 640], bf16, name="dh", tag="dh")
                        nc.vector.tensor_tensor(out=dh[:], in0=t2b[:, 0:640], in1=t2b[:, 640:1280], op=op.add)
                        nc.vector.tensor_tensor(out=dh[:], in0=dh[:], in1=e2v[:, 5120:5760], op=op.add)
                        den = cp1.tile([128, 640], f32, name="den", tag="den")
                        nc.vector.tensor_scalar(out=den[:], in0=dh[:], scalar1=1.0, scalar2=None, op0=op.mult)
                        rs = cp1.tile([128, 640], f32, name="rs", tag="rs")
                        nc.vector.reciprocal_approx_fast(out=rs[:], in_=den[:])

                        # --- numerators: cumsum + strided diff ---
                        cum = cp1.tile([128, 4, CPAD + 1440], f32, name="cum", tag="cum")
                        nc.vector.memset(sb_ap(cum, 0, [[CPAD + 1440, 4], [1, CPAD]]), 0.0)
                        for tag, uf9 in (("c", uf9c), ("i", uf9i)):
                            for qx in range(4):
                                nc.vector._custom_dve(
                                    cops["MUL_CUMSUM_ANT"],
                                    out=sb_ap(cum, qx * (CPAD + 1440) + CPAD, [[1, 1440]]),
                                    in0=sb_ap(e, qx * 1440, [[1, 1440]]),
                                    in1=uf9[:, nn].rearrange("p a b -> p (a b)"))
                            ce = cp1.tile([128, 4, 161], f32, name="ce", tag="ce" + tag)
                            nc.vector.memset(sb_ap(ce, 0, [[161, 4], [1, 1]]), 0.0)
                            nc.scalar.activation(
                                out=sb_ap(ce, 1, [[161, 4], [1, WC]]),
                                in_=sb_ap(cum, CPAD + 8, [[CPAD + 1440, 4], [9, WC]]),
                                func=Act.Copy)
                            acc = cp1.tile([128, 640], f32, name="acc", tag="acc" + tag)
                            nc.vector.tensor_tensor(
                                out=acc[:],
                                in0=sb_ap(ce, 1, [[161, 4], [1, WC]]),
                                in1=sb_ap(ce, 0, [[161, 4], [1, WC]]),
                                op=op.subtract)
                            dst = cu if tag == "c" else iu
                            nc.vector.tensor_tensor(out=dst[:, nn, qy], in0=acc[:], in1=rs[:], op=op.mult)

                # ---------------- fusion (batched) ----------------
                for qy in range(4):
                    dif = cp1.tile([128, 640], f32, name="dif", tag="dif")
                    nc.vector.tensor_tensor(out=dif[:], in0=cu[:, 1, qy], in1=cu[:, 0, qy], op=op.subtract)
                    dm = cp1.tile([128, 640], f32, name="dm", tag="dm")
                    nc.vector.tensor_tensor(out=dm[:], in0=iu[:, 1, qy], in1=iu[:, 0, qy], op=op.subtract)
                    s1 = cp1.tile([128, 640], f32, name="s1", tag="s1")
                    nc.scalar.activation(out=s1[:], in_=dif[:], func=Act.Sigmoid)
                    t = cp1.tile([128, 640], f32, name="t", tag="t")
                    nc.vector.tensor_tensor(out=t[:], in0=s1[:], in1=dm[:], op=op.mult)
                    fus = cp1.tile([128, 640], f32, name="fus", tag="fus")
                    nc.vector.tensor_tensor(out=fus[:], in0=t[:], in1=iu[:, 0, qy], op=op.add)
                    aff = cp1.tile([128, 640], f32, name="aff", tag="aff")
                    nc.scalar.activation(out=aff[:], in_=fus[:], func=Act.Identity,
                                         scale=consts[:, K_DS:K_DS + 1], bias=consts[:, K_DB:K_DB + 1])
                    ot = cp1.tile([128, 160, 4], f32, name="ot", tag="ot")
                    nc.vector.reciprocal_approx_fast(
                        out=ot[:].rearrange("p w q -> p q w"),
                        in_=aff[:].rearrange("p (q w) -> p q w", q=4))
                    nc.sync.dma_start(
                        out=dram_ap(out_d, qy * WC * UP, [[UP * WC * UP, 128], [1, 640]]),
                        in_=ot[:].rearrange("p a b -> p (a b)"))

    nc.finalize()
    return nc


def _host_prep(inputs):
    K_ref = np.asarray(inputs["K_ref"], np.float32)
    K_nei = np.asarray(inputs["K_nei"], np.float32)
    R_nei = np.asarray(inputs["R_nei"], np.float32)
    T_nei = np.asarray(inputs["T_nei"], np.float32)
    depth0 = np.asarray(inputs["depth0"], np.float32)
    flow = np.asarray(inputs["flow"], np.float32)
    mask = np.asarray(inputs["mask"], np.float32)
    conf = np.asarray(inputs["conf"], np.float32)
    dmin = float(np.asarray(inputs["depth_min"]).reshape(-1)[0])
    dmax = float(np.asarray(inputs["depth_max"]).reshape(-1)[0])

    cA = 1.0 / (dmin - dmax)
    cB = -dmax / (dmin - dmax)

    # pixel rays per batch (unit-z), float64 on host
    uv = []
    for b in range(B):
        Ki = np.linalg.inv(K_ref[b, 0, 0].astype(np.float64))
        gx, gy = np.meshgrid(np.arange(W, dtype=np.float64), np.arange(H, dtype=np.float64))
        x = Ki[0, 0] * gx + Ki[0, 1] * gy + Ki[0, 2]
        yy = Ki[1, 0] * gx + Ki[1, 1] * gy + Ki[1, 2]
        z = Ki[2, 0] * gx + Ki[2, 1] * gy + Ki[2, 2]
        uv.append((x / z, yy / z))

    in_maps = []
    for c in range(NCORES):
        b, rh, wh = c // 4, (c // 2) % 2, c % 2
        r0, w0 = rh * RP, wh * WC

        # geometry pixel grid: rows r0..r0+127, cols w0-1..w0+160 (162) +
        # packed halo rows (r0-1, r0+128) x 162 -> cols 162..165
        rows = np.arange(r0, r0 + RP)
        cols = np.clip(np.arange(w0 - 1, w0 + WC + 1), 0, W - 1)  # 162, edge-clamped
        rtop = max(r0 - 1, 0)
        rbot = min(r0 + RP, H - 1)

        pix = np.zeros((128, NN, 2, GX), np.float32)
        geo = np.zeros((128, NN, 9, GX), np.float32)
        hm = np.ones((128, NN, GX), np.float32)
        cbc = np.zeros((128, NN, NCB), np.float32)
        consts = np.zeros((128, NCONST), np.float32)
        consts[:, K_CA] = cA
        consts[:, K_CB] = cB
        consts[:, K_DS] = dmin - dmax
        consts[:, K_DB] = dmax

        ug, vg = uv[b]

        def gather(arr):
            """arr [H, W] -> [128, GX]: main 162 cols + packed halo rows."""
            out = np.zeros((128, GX), np.float32)
            out[:, 0:162] = arr[np.ix_(rows, cols)]
            halo_t = arr[rtop][cols]            # 162
            halo_b = arr[rbot][cols]
            out[0:81, 162:164] = halo_t.reshape(81, 2)
            out[0:81, 164:166] = halo_b.reshape(81, 2)
            return out

        u_g = gather(ug)
        v_g = gather(vg)
        d_g = gather(depth0[b, 0].astype(np.float64))
        for nn in range(NN):
            fl_g = gather(flow[nn, b, 0].astype(np.float64))
            pix[:, nn, 0] = d_g
            pix[:, nn, 1] = fl_g

            Kn = K_nei[nn, b, 0, 0].astype(np.float64)
            Rn = R_nei[nn, b, 0, 0].astype(np.float64)
            Tn = T_nei[nn, b, 0, 0].astype(np.float64).reshape(3)
            M = Kn @ Rn
            t = (Kn @ Tn.reshape(3, 1)).reshape(3)
            iK = np.linalg.inv(Kn)
            assert abs(iK[0, 1]) < 1e-12 and abs(iK[1, 0]) < 1e-12
            assert abs(iK[2, 0]) < 1e-12 and abs(iK[2, 1]) < 1e-12 and abs(iK[2, 2] - 1) < 1e-9

            for j in range(3):
                a_j = M[j, 0] * u_g + M[j, 1] * v_g + M[j, 2]
                geo[:, nn, j] = a_j
                geo[:, nn, 3 + j] = 10.0 * a_j + t[j]
            for j in range(3):
                geo[:, nn, 6 + j] = Rn[j, 0] * u_g + Rn[j, 1] * v_g + Rn[j, 2]

            cbc[:, nn, C_T0] = t[0]
            cbc[:, nn, C_T1] = t[1]
            cbc[:, nn, C_T2] = t[2]
            s = 1.0 + EPS
            cbc[:, nn, C_SA] = iK[0, 0] / s
            cbc[:, nn, C_CA] = iK[0, 2] / s
            cbc[:, nn, C_SB] = iK[1, 1] / s
            cbc[:, nn, C_CB] = iK[1, 2] / s
            cbc[:, nn, C_TXN] = -Tn[0]
            cbc[:, nn, C_TYN] = -Tn[1]
            cbc[:, nn, C_TZ] = Tn[2]

        # hm zeros: invalid halo cols / rows
        if w0 == 0:
            hm[:, :, 0] = 0.0
            hm[0, :, 162] = 0.0   # packed halo rows, left-edge px
            hm[0, :, 164] = 0.0
        if w0 + WC == W:
            hm[:, :, 161] = 0.0
            hm[80, :, 163] = 0.0  # packed halo rows, right-edge px
            hm[80, :, 165] = 0.0
        if r0 == 0:
            hm[:, :, 162:164] = 0.0
        if r0 + RP == H:
            hm[:, :, 164:166] = 0.0
        hm[81:, :, 162:166] = 0.0  # unused packed slots

        confpad = np.zeros((NN, 130, 164), np.float32)
        cw = np.arange(w0 - 1, w0 + WC + 1)
        cwv = (cw >= 0) & (cw < W)
        confpad[:, 1:129, 0:162][:, :, cwv] = conf[:, b, 0, r0:r0 + RP][:, :, cw[cwv]]
        if r0 > 0:
            confpad[:, 0, 0:162][:, cwv] = conf[:, b, 0, r0 - 1][:, cw[cwv]]
        if r0 + RP < H:
            confpad[:, 129, 0:162][:, cwv] = conf[:, b, 0, r0 + RP][:, cw[cwv]]

        # mask: [nn, qy, p, (qx, w, k)] with k = dy*3+dx row-major
        ms = mask[:, b, :, r0:r0 + RP, w0:w0 + WC]          # [NN, 144, 128, 160]
        ms = ms.reshape(NN, 9, 4, 4, RP, WC)               # [NN, k, qy, qx, p, w]
        mask_pk = np.ascontiguousarray(
            ms.transpose(0, 2, 4, 3, 5, 1)                 # [NN, qy, p, qx, w, k]
        ).reshape(NN, 4, 128, 5760).astype(np.float16)
        mask_po = np.ascontiguousarray(
            ms.transpose(0, 2, 4, 1, 3, 5)                 # [NN, qy, p, k, qx, w]
        ).reshape(NN, 4, 128, 9, 640).astype(np.float16)

        in_maps.append({
            "pix": pix, "geo": geo, "hm": hm, "cbc": cbc, "consts": consts,
            "confpad": confpad, "maskpk": mask_pk, "maskpo": mask_po,
        })
    return in_maps


def kernel(**inputs):
    if "nc" not in _cache:
        _cache["nc"] = _build_program()
    nc = _cache["nc"]
    in_maps = _host_prep(inputs)

    from concourse import bass_utils

    res = bass_utils.run_bass_kernel_spmd(nc, in_maps, core_ids=list(range(NCORES)))
    out = np.empty((B, 1, H * UP, W * UP), np.float32)
    for c in range(NCORES):
        b, rh, wh = c // 4, (c // 2) % 2, c % 2
        out[b, 0, rh * RP * UP:(rh + 1) * RP * UP, wh * WC * UP:(wh + 1) * WC * UP] = res.results[c]["out"]
    return out
